# revision 28
# baseline (speedup 1.0000x reference)
"""Trainium2 Bass kernel for BiBo attention (GQA + per-head RMSNorm + RoPE +
SSMax scaling + causal attention + o_proj).

Sharding: tensor-parallel over the 4 KV-head groups x data-parallel over the
2 batch elements = 8 cores. Each core computes its 4 q-heads / 1 kv-head of
attention for one batch element plus its row-slice of o_proj; the host sums
the 4 partial o_proj outputs per batch element (row-parallel unshard).

Layout strategy (per core):
  - hidden^T [H, S] streamed from DRAM; projections produce q^T/k^T with the
    head dim on partitions so QK^T needs no transposes.
  - scores are computed transposed (scoresT[k, q]) so the PV matmul consumes
    exp(scoresT) directly; no max-subtraction is needed because RMS-normed
    q/k bound |scores| <= sqrt(HD)*ssmax*log(S) ~ 10.
  - block-sparse causal skipping: mask blocks entirely <= -1e8 are skipped;
    exact causal-diagonal blocks restrict QK/exp/PV to the visible
    q-subrange and add a [128,128] triangular mask tile on DVE; other
    step/mask blocks fall back to the generic one-hot / additive paths.
  - softmax denominator: exp tiles tree-summed (DVE pairs, gpsimd second
    level) so only a few ones-matmuls hit the PE per (head, q-tile).
"""

import math

import numpy as np

B, S, H = 2, 2048, 2048
NH, NKV, HD = 16, 4, 128
EPS = 1e-6
NCORES = 8
TP = 4            # kv-head groups
QH = NH // NKV    # q heads per core
SC = 512          # q-tile / s-chunk width
NSC = S // SC     # 4
KT = 128          # k tile
NKT = S // KT     # 16
HC = 128          # h contraction chunk
NHC = H // HC     # 16
SKIP_THRESH = -1e8

_compiled_cache = {}
LAST_EXEC_NS = None
LAST_RESULTS = None


def _build_program(plan, mask_counts):
    import concourse.mybir as mybir
    import concourse.tile as tile
    from concourse import bacc

    F32 = mybir.dt.float32
    MM = mybir.dt.bfloat16
    AF = mybir.ActivationFunctionType
    OP = mybir.AluOpType

    n_mask = sum(mask_counts)

    nc = bacc.Bacc("TRN2", target_bir_lowering=False, debug=False,
                   num_devices=NCORES)
    hT = nc.dram_tensor("hT", [NSC, 4, HC, 4 * SC], MM,
                        kind="ExternalInput").ap()
    wqT = nc.dram_tensor("wqT", [4, HC, 4 * QH * HD], MM,
                         kind="ExternalInput").ap()
    wkvT = nc.dram_tensor("wkvT", [2, HC, 8 * 2 * HD], MM,
                          kind="ExternalInput").ap()
    woT = nc.dram_tensor("woT", [QH * HD, H], MM, kind="ExternalInput").ap()
    csT = nc.dram_tensor("csT", [NSC, HD, 2 * SC], F32,
                         kind="ExternalInput").ap()
    qsc = nc.dram_tensor("qsc", [1, QH], F32, kind="ExternalInput").ap()
    qsb = nc.dram_tensor("qsb", [1, QH], F32, kind="ExternalInput").ap()
    iwq = nc.dram_tensor("iwq", [HD, 1], MM, kind="ExternalInput").ap()
    iwk = nc.dram_tensor("iwk", [HD, 1], MM, kind="ExternalInput").ap()
    trib = nc.dram_tensor("trib", [KT, KT], MM, kind="ExternalInput").ap()
    idn = nc.dram_tensor("idn", [KT, KT], MM, kind="ExternalInput").ap()
    if n_mask:
        mblk = nc.dram_tensor("mblk", [n_mask, KT, SC], F32,
                              kind="ExternalInput").ap()
        mtri = nc.dram_tensor("mtri", [KT, KT], MM, kind="ExternalInput").ap()
        mhot = nc.dram_tensor("mhot", [n_mask, KT, SC], MM,
                              kind="ExternalInput").ap()
    out = nc.dram_tensor("out", [S, H], MM, kind="ExternalOutput").ap()

    with tile.TileContext(nc) as tc:
        _emit(nc, tc, locals(), plan, mask_counts, MM, F32, AF, OP)
    nc.compile()
    return nc


def _emit(nc, tc, T, plan, mask_counts, MM, F32, AF, OP):
    from contextlib import ExitStack

    hT, wqT, wkvT, woT = T["hT"], T["wqT"], T["wkvT"], T["woT"]
    csT = T["csT"]
    qsc, qsb = T["qsc"], T["qsb"]
    iwq, iwk, out = T["iwq"], T["iwk"], T["out"]
    trib = T["trib"]
    idn = T["idn"]
    mblk = T.get("mblk")
    mtri = T.get("mtri")
    mhot = T.get("mhot")

    ctx = ExitStack()
    with ctx:
        const = ctx.enter_context(tc.tile_pool(name="const", bufs=1))
        wpool = ctx.enter_context(tc.tile_pool(name="w", bufs=1))
        persist = ctx.enter_context(tc.tile_pool(name="persist", bufs=1))
        hpool = ctx.enter_context(tc.tile_pool(name="h", bufs=6))
        mpool = ctx.enter_context(tc.tile_pool(name="m", bufs=6))
        spool = ctx.enter_context(tc.tile_pool(name="s", bufs=2))
        epool = ctx.enter_context(tc.tile_pool(name="e", bufs=3))
        atpool = ctx.enter_context(tc.tile_pool(name="at", bufs=8))
        opool_sb = ctx.enter_context(tc.tile_pool(name="osb", bufs=2))
        ps_mm = ctx.enter_context(tc.tile_pool(name="psmm", bufs=3, space="PSUM"))
        ps_pv = ctx.enter_context(tc.tile_pool(name="pspv", bufs=2, space="PSUM"))
        ps_es = ctx.enter_context(tc.tile_pool(name="pses", bufs=1, space="PSUM"))
        ps_o = ctx.enter_context(tc.tile_pool(name="pso", bufs=2, space="PSUM"))

        # ---- persistent tiles (loads emitted by the driver below) -------
        wq_g = [wpool.tile([128, 4 * QH * HD], MM, name=f"wqg{g}",
                           tag=f"wqg{g}") for g in range(4)]
        wkv_g = [wpool.tile([128, 8 * 2 * HD], MM, name=f"wkvg{g}",
                            tag=f"wkvg{g}") for g in range(2)]
        wo_t = wpool.tile([128, QH * H], MM, tag="wo")
        wq_ts = [wq_g[c // 4][:, (c % 4) * QH * HD:(c % 4 + 1) * QH * HD]
                 for c in range(NHC)]
        wkv_ts = [wkv_g[c // 8][:, (c % 8) * 2 * HD:(c % 8 + 1) * 2 * HD]
                  for c in range(NHC)]
        cs_t = wpool.tile([128, NSC * 2 * SC], F32, tag="cs")
        cs_loaded = [False] * NSC

        def cos_sl(sc):
            return cs_t[:, 2 * sc * SC:(2 * sc + 1) * SC]

        def sin_sl(sc):
            return cs_t[:, (2 * sc + 1) * SC:(2 * sc + 2) * SC]

        def load_cs(sc):
            if not cs_loaded[sc]:
                nc.scalar.dma_start(cs_t[:, 2 * sc * SC:(2 * sc + 2) * SC],
                                    csT[sc])
                cs_loaded[sc] = True
        qsc_t = const.tile([1, QH], F32, tag="qsc")
        qsb_t = const.tile([1, QH], F32, tag="qsb")
        iwq_t = const.tile([128, 1], MM, tag="iwq")
        iwk_t = const.tile([128, 1], MM, tag="iwk")
        ones_t = const.tile([128, 1], MM, tag="ones")
        eps_t = const.tile([1, 1], F32, tag="eps")
        trib_t = const.tile([128, KT], MM, tag="trib", name="trib")
        idn_t = const.tile([128, KT], MM, tag="idn", name="idn")
        tri_t = (const.tile([128, KT], MM, tag="tri", name="tri")
                 if mtri is not None else None)
        khat = [persist.tile([128, SC], MM, name=f"khat{j}", tag=f"khat{j}")
                for j in range(NSC)]
        v_sb = [persist.tile([128, SC], MM, name=f"v{j}", tag=f"v{j}")
                for j in range(NSC)]
        qhat = [[persist.tile([128, SC], MM, name=f"qhat{i}_{j}",
                              tag=f"qhat{i}_{j}") for j in range(NSC)]
                for i in range(QH)]

        def load_wkv():
            for g in range(2):
                nc.sync.dma_start(wkv_g[g][:], wkvT[g])

        def load_early():
            # scalar (ACT HWDGE) ring: consts, cs, then wq; drains in
            # parallel with the sync ring across the shared SDMA engines.
            nc.scalar.dma_start(qsc_t[:], qsc[:])
            nc.scalar.dma_start(qsb_t[:], qsb[:])
            nc.scalar.dma_start(iwq_t[:], iwq[:])
            nc.scalar.dma_start(iwk_t[:], iwk[:])
            nc.scalar.dma_start(trib_t[:], trib[:])
            nc.scalar.dma_start(idn_t[:], idn[:])
            load_cs(0)
            for g in range(4):
                nc.scalar.dma_start(wq_g[g][:], wqT[g])
            nc.vector.memset(ones_t[:], 1.0)
            nc.vector.memset(eps_t[:], EPS)
            if mtri is not None:
                nc.scalar.dma_start(tri_t[:], mtri[:])

        def load_wo():
            for f in range(QH):
                nc.scalar.dma_start(wo_t[:, f * H:(f + 1) * H],
                                    woT[f * HD:(f + 1) * HD, :])

        # norm+rope staged: s1 (right after the proj matmuls) does the
        # square on ACT straight off PSUM, the cos-product, and the rotated
        # sin-product as two half-height muls (the sign is folded into the
        # host-side sin table); the var matmul (s2) trails by one projection
        # group; s3 finishes rstd + the rope sum off the PSUM path.
        def norm_s1(pp, sc):
            sq = spool.tile([128, SC], MM, tag="sq", name="sq")
            nc.scalar.square(sq[:], pp[:])
            uu = spool.tile([128, SC], MM, tag="uu", name="uu")
            nc.vector.tensor_mul(uu[:], pp[:], cos_sl(sc))
            tt = spool.tile([128, SC], MM, tag="tt", name="tt")
            sn = sin_sl(sc)
            nc.vector.tensor_mul(tt[0:64, :], pp[64:128, :], sn[0:64, :])
            nc.vector.tensor_mul(tt[64:128, :], pp[0:64, :], sn[64:128, :])
            return tt, sq, uu

        def norm_s2(sq, iw_t):
            var = ps_mm.tile([1, SC], F32, tag="mm", name="var")
            nc.tensor.matmul(var[:], iw_t[:], sq[:], start=True, stop=True)
            return var

        def norm_s3(tt, uu, var, sc, hd, hat_dst):
            # sd = sqrt(var + eps)/qc via folded scale/bias; rs = qc/rms
            sd = spool.tile([1, SC], F32, tag="sd", name="sd")
            if hd is None:
                nc.scalar.activation(sd[:], var[:], AF.Sqrt, bias=eps_t[:])
            else:
                nc.scalar.activation(sd[:], var[:], AF.Sqrt,
                                     bias=qsb_t[:, hd:hd + 1],
                                     scale=qsc_t[:, hd:hd + 1])
            rs = spool.tile([1, SC], F32, tag="rs", name="rs")
            nc.vector.reciprocal_approx_fast(rs[:], sd[:])
            bb = spool.tile([128, SC], F32, tag="bb", name="bb")
            nc.gpsimd.partition_broadcast(bb[:], rs[:], 128)
            nc.vector.tensor_add(tt[:], tt[:], uu[:])
            nc.vector.tensor_mul(hat_dst, tt[:], bb[:])

        # ---- projections, per s-chunk -----------------------------------
        def hts_load(sc, first=False):
            tiles = []
            for g in range(4):
                t = hpool.tile([128, 4 * SC], MM, tag="ht", name="ht")
                eng = nc.scalar if (first and g == 0) else nc.sync
                eng.dma_start(t[:], hT[sc, g])
                tiles.append(t)
            return [tiles[c // 4][:, (c % 4) * SC:(c % 4 + 1) * SC]
                    for c in range(NHC)]

        def proj_chunk(sc, hts):
            # five projections (k, q0..q3), staged so each var matmul is
            # emitted after the NEXT projection's matmul group
            specs = [(iwk_t, None, khat[sc])] + [
                (iwq_t, hd, qhat[hd][sc]) for hd in range(QH)]
            state = []  # (sh, sq, uu, spec)

            def do_mm(idx):
                pp = ps_mm.tile([128, SC], F32, tag="mm", name="pp")
                for c in range(NHC):
                    if idx == 0:
                        w_sl = wkv_ts[c][:, 0:HD]
                    else:
                        w_sl = wq_ts[c][:, (idx - 1) * HD:idx * HD]
                    nc.tensor.matmul(pp[:], w_sl, hts[c][:],
                                     start=(c == 0), stop=(c == NHC - 1))
                tt, sq, uu = norm_s1(pp, sc)
                state.append((tt, sq, uu, specs[idx]))

            def finish_one():
                tt, sq, uu, (iw_t, hd, dst) = state.pop(0)
                var = norm_s2(sq, iw_t)
                norm_s3(tt, uu, var, sc, hd, dst[:])

            do_mm(0)
            for idx in range(1, 5):
                do_mm(idx)
                finish_one()
            finish_one()
            # v-proj: natural [s, d] layout, N=128 matmuls
            for ss in range(4):
                vp = ps_o.tile([128, SC], F32, tag="o", name="vp")
                for c in range(NHC):
                    nc.tensor.matmul(vp[:, 0:HD],
                                     hts[c][:, ss * 128:(ss + 1) * 128],
                                     wkv_ts[c][:, HD:2 * HD],
                                     start=(c == 0), stop=(c == NHC - 1))
                nc.scalar.copy(v_sb[sc][:, ss * 128:(ss + 1) * 128],
                               vp[:, 0:HD])

        # ---- attention + o_proj, per q-tile ------------------------------
        mask_starts = [sum(mask_counts[:i]) for i in range(NSC)]

        def attn_qtile(qi):
            mask_idx = mask_starts[qi]
            kts = [kt for kt in range(NKT) if plan[qi][kt] != "skip"]
            # load this q-tile's mask blocks (shared across heads)
            mtiles = {}
            for kt in kts:
                kind = plan[qi][kt]
                if kind == "diag":
                    mtiles[kt] = ("diag", None)
                elif kind == "step":
                    mt = mpool.tile([128, SC], MM, tag="maskh", name="mh")
                    nc.gpsimd.dma_start(mt[:], mhot[mask_idx])
                    mtiles[kt] = ("step", mt)
                    mask_idx += 1
                elif kind == "mask":
                    mt = mpool.tile([128, SC], F32, tag="mask", name="mk")
                    nc.sync.dma_start(mt[:], mblk[mask_idx])
                    mtiles[kt] = ("mask", mt)
                    mask_idx += 1

            def off_of(kt):
                kind, _ = mtiles.get(kt, (None, None))
                if kind == "diag":
                    return kt * KT - qi * SC
                return 0

            ats = []
            for hd in range(QH):
                qsl = qhat[hd][qi]
                pv = ps_pv.tile([128, SC], F32, tag="pv")
                es = ps_es.tile([1, SC], F32, tag="es")
                sts = {}
                # acc[0]: pending full tile; acc[1]: L1 pair-sums pending
                # gpsimd L2; acc[2]: es-MM survivors; acc[3]: diag partials
                acc = ([], [], [], [])
                # pipeline QK^T one k-tile ahead of exp/PV
                for j, kt in enumerate(kts):
                    st = ps_mm.tile([128, SC], F32, tag="mm")
                    kind, mt = mtiles.get(kt, (None, None))
                    off = off_of(kt)
                    nc.tensor.matmul(st[:, off:SC],
                                     khat[kt // 4][:, (kt % 4) * 128:
                                                   (kt % 4 + 1) * 128],
                                     qsl[:, off:SC],
                                     start=True,
                                     stop=(kind not in ("step", "diag")))
                    if kind == "diag":
                        nc.tensor.matmul(st[:, off:off + KT], trib_t[:],
                                         idn_t[:], start=False, stop=True)
                    elif kind == "step":
                        nc.tensor.matmul(st[:], tri_t[:], mt[:],
                                         start=False, stop=True)
                    elif kind == "mask":
                        nc.vector.tensor_add(st[:], st[:], mt[:])
                    sts[j] = st
                    if j >= 1:
                        _attn_tail(nc, j - 1, kts, sts, off_of, pv, v_sb,
                                   MM, AF, epool, acc)
                _attn_tail(nc, len(kts) - 1, kts, sts, off_of, pv, v_sb,
                           MM, AF, epool, acc)
                _es_reduce(nc, es, acc, ones_t, MM, epool)
                rs = spool.tile([1, SC], F32, tag="ars")
                nc.vector.reciprocal_approx_fast(rs[:], es[:])
                bb = spool.tile([128, SC], F32, tag="abb")
                nc.gpsimd.partition_broadcast(bb[:], rs[:], 128)
                at = atpool.tile([128, SC], MM, tag="at")
                nc.vector.tensor_mul(at[:, 0:SC // 2], pv[:, 0:SC // 2],
                                     bb[:, 0:SC // 2])
                nc.vector.tensor_mul(at[:, SC // 2:], pv[:, SC // 2:],
                                     bb[:, SC // 2:])
                ats.append(at)
            # o_proj for this q-tile
            for ss in range(4):
                ob = opool_sb.tile([128, H], MM, tag="osb", name="ob")
                for ho in range(4):
                    op_t = ps_o.tile([128, SC], F32, tag="o", name="op")
                    for hd in range(QH):
                        nc.tensor.matmul(
                            op_t[:],
                            ats[hd][:, ss * 128:(ss + 1) * 128],
                            wo_t[:, hd * H + ho * SC:hd * H + (ho + 1) * SC],
                            start=(hd == 0), stop=(hd == QH - 1))
                    nc.vector.tensor_copy(ob[:, ho * SC:(ho + 1) * SC],
                                          op_t[:])
                    nc.sync.dma_start(
                        out[qi * SC + ss * 128:qi * SC + (ss + 1) * 128,
                            ho * SC:(ho + 1) * SC],
                        ob[:, ho * SC:(ho + 1) * SC])

        # ---- driver: software-pipelined phase order ----------------------
        load_wkv()
        hts0 = hts_load(0, first=True)
        load_early()
        proj_chunk(0, hts0)
        hts1 = hts_load(1)
        load_cs(1)
        proj_chunk(1, hts1)
        load_wo()
        hts2 = hts_load(2)
        load_cs(2)
        attn_qtile(0)
        hts3 = hts_load(3)
        load_cs(3)
        attn_qtile(1)
        proj_chunk(2, hts2)
        attn_qtile(2)
        proj_chunk(3, hts3)
        attn_qtile(3)


def _attn_tail(nc, j, kts, sts, off_of, pv, v_sb, MM, AF, epool, acc):
    """exp + PV per k-tile; exp tiles tree-summed inline for the softmax
    denominator: full-width tiles pair on DVE (L1), pair-sums pair on
    gpsimd (L2), diagonal subrange tiles chain right-aligned."""
    pend, l2p, survivors, dtiles = acc
    kt = kts[j]
    st = sts.pop(j)
    off = off_of(kt)
    ex = epool.tile([128, SC], MM, tag="ex", name="ex", bufs=8)
    nc.scalar.activation(ex[:, off:SC], st[:, off:SC], AF.Exp)
    last = j == len(kts) - 1
    nc.tensor.matmul(pv[:, off:SC],
                     v_sb[kt // 4][:, (kt % 4) * 128:(kt % 4 + 1) * 128],
                     ex[:, off:SC], start=(j == 0), stop=last)
    if off:
        if dtiles:
            a, ao = dtiles[0]
            nc.vector.tensor_add(a[:, off:SC], a[:, off:SC], ex[:, off:SC])
        else:
            dtiles.append((ex, off))
        return
    pend.append(ex)
    if len(pend) == 2:
        a, b = pend
        pend.clear()
        sm = epool.tile([128, SC], MM, tag="exs", name="exs", bufs=4)
        nc.vector.tensor_add(sm[:], a[:], b[:])
        l2p.append(sm)
        if len(l2p) == 2:
            c, d = l2p
            l2p.clear()
            sg = epool.tile([128, SC], MM, tag="exg", name="exg", bufs=4)
            nc.vector.tensor_add(sg[:], c[:], d[:])
            survivors.append(sg)


def _es_reduce(nc, es, acc, ones_t, MM, epool):
    """Fold leftovers and fire the accumulating ones-matmuls."""
    pend, l2p, survivors, dtiles = acc
    survivors = survivors + l2p + pend
    if dtiles:
        a, ao = dtiles[0]
        # merge the right-aligned diag partial into a full-width survivor
        f = survivors[0]
        nc.vector.tensor_add(f[:, ao:SC], f[:, ao:SC], a[:, ao:SC])
    while len(survivors) > 2:
        b = survivors.pop()
        a = survivors.pop()
        sm = epool.tile([128, SC], MM, tag="exf", name="exf", bufs=2)
        nc.vector.tensor_add(sm[:], a[:], b[:])
        survivors.append(sm)
    for i, sm in enumerate(survivors):
        nc.tensor.matmul(es[:], ones_t[:], sm[:],
                         start=(i == 0), stop=(i == len(survivors) - 1))


def _is_step(blk):
    """True if every batch/column is 0 for k < f and exactly -1e9 for k >= f."""
    isneg = blk == np.float32(-1e9)
    iszero = blk == 0.0
    if not (isneg | iszero).all():
        return False
    # per (b, q): suffix property along k
    f = isneg.argmax(axis=-1) + 0  # first masked k (0 if none masked)
    any_neg = isneg.any(axis=-1)
    kk = np.arange(blk.shape[-1])
    want = np.where(any_neg[..., None], kk[None, None] >= f[..., None], False)
    return bool((isneg == want).all())


def _is_causal_diag(blk, qi, kt):
    """True if the block is exactly the standard causal mask: masked iff
    kt*KT + k > qi*SC + q, and the block starts at or right of the q-tile
    (so the invisible region is a clean left subrange)."""
    off = kt * KT - qi * SC
    if off < 0 or off + KT > SC:
        return False
    kk = np.arange(KT)[None, :]
    qq = np.arange(SC)[:, None]
    want = (kt * KT + kk) > (qi * SC + qq)  # [q, k] masked
    isneg = blk == np.float32(-1e9)
    iszero = blk == 0.0
    if not (isneg | iszero).all():
        return False
    return bool((isneg == want[None]).all())


def _mask_plan(mask):
    """Classify [qi][kt] blocks of the (q,k) mask, unified across batch."""
    plan = []
    for qi in range(NSC):
        row = []
        for kt in range(NKT):
            blk = mask[:, 0, qi * SC:(qi + 1) * SC, kt * KT:(kt + 1) * KT]
            if (blk <= SKIP_THRESH).all():
                row.append("skip")
            elif (blk == 0.0).all():
                row.append("zero")
            elif _is_causal_diag(blk, qi, kt):
                row.append("diag")
            elif _is_step(blk):
                row.append("step")
            else:
                row.append("mask")
        # guard: a q-tile with no included block would divide by zero
        if all(s == "skip" for s in row):
            row[0] = "mask"
        plan.append(row)
    return plan


def kernel(hidden_states, cos, sin, attention_mask, wq, wk, wv, wo,
           q_norm_w, k_norm_w, ssmax_scale):
    global LAST_EXEC_NS
    import os
    import ml_dtypes
    from concourse.bass_utils import run_bass_kernel_spmd

    f32 = np.float32
    hidden_states = np.asarray(hidden_states, f32)
    cos = np.asarray(cos, f32)
    sin = np.asarray(sin, f32)
    attention_mask = np.asarray(attention_mask, f32)
    wq = np.asarray(wq, f32)
    wk = np.asarray(wk, f32)
    wv = np.asarray(wv, f32)
    wo = np.asarray(wo, f32)
    q_norm_w = np.asarray(q_norm_w, f32)
    k_norm_w = np.asarray(k_norm_w, f32)
    ssmax = np.asarray(ssmax_scale, f32).reshape(NH)

    plan = _mask_plan(attention_mask)
    mask_counts = [sum(1 for s in row if s in ("mask", "step")) for row in plan]
    key = (tuple(tuple(r) for r in plan),)
    if key not in _compiled_cache:
        _compiled_cache[key] = _build_program(plan, mask_counts)
    nc = _compiled_cache[key]

    bf16 = ml_dtypes.bfloat16
    qw = np.tile(q_norm_w, QH)
    iwq_np = (1.0 / (HD * q_norm_w ** 2)).astype(bf16)[:, None]
    iwk_np = (1.0 / (HD * k_norm_w ** 2)).astype(bf16)[:, None]
    sinT = sin.T.copy()
    sinT[0:64] = -sinT[0:64]   # fold rotate_half's sign into the table
    cs_np = np.concatenate(
        [cos.T.reshape(HD, NSC, 1, SC), sinT.reshape(HD, NSC, 1, SC)],
        axis=2)  # [HD, NSC, 2, SC]
    cs_np = np.ascontiguousarray(
        cs_np.transpose(1, 0, 2, 3).reshape(NSC, HD, 2 * SC))
    # trib[j, k] = -1e9 * [k > j]  (lhsT layout: TRI^T so TRI^T.T@I = TRI)
    trib_np = (-1e9 * (np.arange(KT)[None, :] > np.arange(KT)[:, None])
               ).astype(bf16)
    idn_np = np.eye(KT, dtype=np.float32).astype(bf16)

    in_maps = []
    for core in range(NCORES):
        b, g = divmod(core, TP)
        hTm = np.ascontiguousarray(
            hidden_states[b].T.reshape(4, 4, HC, NSC, SC)
            .transpose(3, 0, 2, 1, 4).reshape(NSC, 4, HC, 4 * SC)
        ).astype(bf16)
        wq_s = wq[g * QH * HD:(g + 1) * QH * HD] * qw[:, None]
        wk_s = wk[g * HD:(g + 1) * HD] * k_norm_w[:, None]
        wv_s = wv[g * HD:(g + 1) * HD]
        wo_s = wo[:, g * QH * HD:(g + 1) * QH * HD]
        qcv = np.array([ssmax[g * QH + i] * math.log(S) / math.sqrt(HD)
                        for i in range(QH)], f32)
        qsc_np = (1.0 / qcv ** 2)[None, :].astype(f32)
        qsb_np = (EPS / qcv ** 2)[None, :].astype(f32)
        wqTm = np.ascontiguousarray(
            wq_s.T.reshape(4, 4, HC, QH * HD)
            .transpose(0, 2, 1, 3).reshape(4, HC, 4 * QH * HD)).astype(bf16)
        wkv = np.concatenate(
            [wk_s.T.reshape(NHC, HC, 1, HD), wv_s.T.reshape(NHC, HC, 1, HD)],
            axis=2)  # [NHC, HC, 2, HD]
        wkvm = np.ascontiguousarray(
            wkv.reshape(2, 8, HC, 2 * HD).transpose(0, 2, 1, 3)
            .reshape(2, HC, 8 * 2 * HD)).astype(bf16)
        m = {
            "hT": hTm,
            "wqT": wqTm,
            "wkvT": wkvm,
            "woT": np.ascontiguousarray(wo_s.T).astype(bf16),
            "csT": cs_np,
            "qsc": qsc_np, "qsb": qsb_np, "iwq": iwq_np, "iwk": iwk_np,
            "trib": trib_np, "idn": idn_np,
        }
        n_mask = sum(mask_counts)
        if n_mask:
            blocks = np.zeros((n_mask, KT, SC), f32)
            hots = np.zeros((n_mask, KT, SC), f32)
            i = 0
            for qi in range(NSC):
                for kt in range(NKT):
                    kind = plan[qi][kt]
                    if kind not in ("mask", "step"):
                        continue
                    blkT = attention_mask[
                        b, 0, qi * SC:(qi + 1) * SC,
                        kt * KT:(kt + 1) * KT].T
                    if kind == "mask":
                        blocks[i] = blkT
                    else:
                        isneg = blkT == np.float32(-1e9)
                        f = isneg.argmax(axis=0)
                        anyneg = isneg.any(axis=0)
                        qsel = np.nonzero(anyneg)[0]
                        hots[i][f[qsel], qsel] = 1.0
                    i += 1
            m["mblk"] = blocks
            m["mhot"] = hots.astype(bf16)
            # tri[r, k] = -1e9 * [k >= r]; lhsT layout [r(part), k(free)]
            tri = (-1e9 * (np.arange(KT)[None, :] >= np.arange(KT)[:, None]))
            m["mtri"] = np.ascontiguousarray(tri).astype(bf16)
        in_maps.append(m)

    trace = bool(int(os.environ.get("BASS_KERNEL_TRACE", "0")))
    res = run_bass_kernel_spmd(nc, in_maps, list(range(NCORES)), trace=trace)
    LAST_EXEC_NS = res.exec_time_ns
    globals()["LAST_RESULTS"] = res

    final = np.zeros((B, S, H), f32)
    for core in range(NCORES):
        b = core // TP
        final[b] += res.results[core]["out"].astype(f32)
    return final
